# revision 15
# baseline (speedup 1.0000x reference)
"""Trainium2 Bass kernel for nn_Cross_Attention_18425409700231.

Per-sample channel attention (16 heads x 8 channels, L2-normalized over
spatial, softmax over in-head channels) followed by a conv block
(3x3 conv -> LeakyReLU -> 1x1 conv -> reflect-pad depthwise 3x3 ->
LeakyReLU, plus 1x1 shortcut) and a residual add.

Sharding: data-parallel over batch B=8 -> one sample per NeuronCore.

Device algorithm per core (sample b), all layouts [C=128 partitions, H*W]:
  A. Gram matrix G = x1 @ x1^T (contract over 16384 spatial) via
     PE-transposed bf16 chunks; norms from diag(G); S = rn*G*rn (one PE
     transpose for the column scale, exploiting symmetry); E = exp(S*temp)
     masked to the 16 block-diagonal 8x8 head blocks; rinv = 1/rowsum.
  B. Attention is folded into the conv weights: L_t = E diag(rinv) w1_t
     and Lsc = E diag(rinv) wsc, so conv1(P) = sum_t L_t^T @ x2_shift and
     sc = Lsc^T @ x2 with P never materialized.
  C. conv1 3x3 evaluated in fp8 (e4m3) with DoubleRow perf mode: taps are
     processed two-at-a-time per matmul (k-tiles over a 4-dim overlapping
     AP into a zero-padded fp8 image supplied pre-padded by the host),
     0.5 PE cycles/row. bias+LeakyReLU fused in the PSUM->SBUF ACT copy,
     written as fp8 into a reflect-padded buffer.
  D. conv2(1x1)+depthwise 3x3 fused into 9 taps of modified weights
     W2t[t] = dw_w[:,t] * conv2_w (host-quantized fp8, x128), evaluated
     the same DoubleRow way; shortcut 1x1 matmul in bf16; final
     out = lrelu(.) + (sc + sc_b) + x1 on DVE, streamed to DRAM as fp16.
  C/D are emitted interleaved per 4-row band so the PE never drains.
"""

import numpy as np
import ml_dtypes

B, C, H, W = 8, 128, 128, 128
HW = H * W
HEADS, HEAD_C = 16, 8
SLOPE = 0.2
EPS = 1e-12
PW = W + 2  # padded width
NB = H // 4  # 32 bands of 4 rows
SL = 16.0   # fp8 scale for attention-fused conv1 weights
SW2 = 128.0  # fp8 scale for fused conv2*dw weights
# tap order: taps (dy,dx) indexed t=dy*3+dx, arranged in DoubleRow pairs
# (base_dy, base_dx, kt_stride): kt0 at base, kt1 at base+stride
TAP_ORDER = [0, 1, 3, 4, 6, 7, 2, 5, None, 8]  # None -> zero weights
PAIR_GEO = [(0, 0, 1), (1, 0, 1), (2, 0, 1), (0, 2, PW), (1, 2, PW)]

_cache = {}


def _build_program(debug=False):
    import concourse.bass as bass
    import concourse.tile as tile
    import concourse.mybir as mybir
    from concourse import bacc
    from concourse.ap import AP

    dt = mybir.dt
    f32, f32r, bf16 = dt.float32, dt.float32r, dt.bfloat16
    fp8, fp16 = dt.float8e4, dt.float16
    AF = mybir.ActivationFunctionType
    ALU = mybir.AluOpType
    AX = mybir.AxisListType
    PM = mybir.MatmulPerfMode

    nc = bacc.Bacc("TRN2", num_devices=8)

    x1h = nc.dram_tensor("x1h", [C, HW], bf16, kind="ExternalInput").ap()
    x2h = nc.dram_tensor("x2h", [C, HW], bf16, kind="ExternalInput").ap()
    x2f8 = nc.dram_tensor("x2f8", [C, (H + 2) * PW], fp8, kind="ExternalInput").ap()
    wc1 = nc.dram_tensor("wc1", [C, 10, C], bf16, kind="ExternalInput").ap()
    wc2 = nc.dram_tensor("wc2", [C, 10, C], fp8, kind="ExternalInput").ap()
    wsc = nc.dram_tensor("wsc", [C, C], f32r, kind="ExternalInput").ap()
    scl = nc.dram_tensor("scl", [C, 7], f32, kind="ExternalInput").ap()
    bmask = nc.dram_tensor("bmask", [C, C], f32, kind="ExternalInput").ap()
    idf = nc.dram_tensor("idf", [C, C], f32, kind="ExternalInput").ap()
    idb = nc.dram_tensor("idb", [C, C], bf16, kind="ExternalInput").ap()
    out = nc.dram_tensor("out", [C, HW], fp16, kind="ExternalOutput").ap()
    if debug:
        dbg_em = nc.dram_tensor("dbg_em", [C, C], f32, kind="ExternalOutput").ap()
        dbg_rinv = nc.dram_tensor("dbg_rinv", [C, 1], f32, kind="ExternalOutput").ap()
        dbg_ph = nc.dram_tensor("dbg_ph", [C, (H + 2) * PW], f32, kind="ExternalOutput").ap()

    def dr_rhs(buf, row, col, kts):
        # [C][kt=2,kts][4 rows,PW][128 cols,1] overlapping window AP
        return AP(buf.tensor, buf.offset + row * PW + col,
                  [list(buf.ap[0]), [kts, 2], [PW, 4], [1, C]])

    with tile.TileContext(nc) as tc:
        with (
            tc.tile_pool(name="consts", bufs=1) as consts,
            tc.tile_pool(name="pads", bufs=1) as pads,
            tc.tile_pool(name="attn", bufs=1) as attn,
            tc.tile_pool(name="streams", bufs=2) as streams,
            tc.tile_pool(name="bands", bufs=3) as bands,
        ):
            # ---- constants to SBUF ----
            idbs = consts.tile([C, C], bf16)
            nc.gpsimd.dma_start(out=idbs, in_=idb)
            idfs = consts.tile([C, C], f32)
            nc.gpsimd.dma_start(out=idfs, in_=idf)
            scls = consts.tile([C, 7], f32)
            nc.gpsimd.dma_start(out=scls, in_=scl)
            bmasks = consts.tile([C, C], f32)
            nc.gpsimd.dma_start(out=bmasks, in_=bmask)
            w1s = consts.tile([C, 10, C], bf16)
            nc.gpsimd.dma_start(out=w1s, in_=wc1)
            w2s = consts.tile([C, 10, C], fp8)
            nc.gpsimd.dma_start(out=w2s, in_=wc2)
            wscs = consts.tile([C, C], f32r)
            nc.gpsimd.dma_start(out=wscs, in_=wsc)
            b1_ap = scls[:, 0:1]
            b2_ap = scls[:, 1:2]
            bsc_ap = scls[:, 2:3]
            temp_ap = scls[:, 3:4]
            c16_ap = scls[:, 4:5]
            cinvSL_ap = scls[:, 5:6]
            cinvW2_ap = scls[:, 6:7]

            # ---- persistent buffers ----
            x1s = pads.tile([C, HW], bf16)      # full x1 (gram + residual)
            x2s = pads.tile([C, HW], bf16)      # full x2 (shortcut)
            p2xf8 = pads.tile([C, H + 2, PW], fp8)  # host-prepadded fp8 x2
            phf8 = pads.tile([C, H + 2, PW], fp8)   # conv1 out, reflect-pad

            # ================= phase A: Gram + softmax =================
            with (
                tc.tile_pool(name="psG", bufs=1, space="PSUM") as psG,
                tc.tile_pool(name="psT", bufs=3, space="PSUM") as psT,
                tc.tile_pool(name="psWarm", bufs=1, space="PSUM") as psWarm,
            ):
                warm = psWarm.tile([C, C], bf16, name="warm")

                def pe_warmup(n):
                    # dummy transposes keep the PE p-state ramp hot while it
                    # waits on serial DVE/ACT chains
                    for i in range(n):
                        nc.tensor.transpose(out=warm, in_=idbs, identity=idbs)

                gps = psG.tile([C, C], f32)
                kk = 0
                col0 = 0
                for ncols in (256, 768, 1024, 2048, 4096, 4096, 4096):
                    nc.sync.dma_start(
                        out=x1s[:, col0 : col0 + ncols],
                        in_=x1h[:, col0 : col0 + ncols],
                    )
                    for g in range(ncols // 512):  # batches of 4 chunks of 128
                        tp = psT.tile([C, 4, C], bf16)
                        for i in range(4):
                            k = (col0 // 512 + g) * 4 + i
                            nc.tensor.transpose(
                                out=tp[:, i, :],
                                in_=x1s[:, k * 128 : (k + 1) * 128],
                                identity=idbs,
                            )
                        tsb = streams.tile([C, 4, C], bf16, bufs=4)
                        nc.vector.tensor_copy(out=tsb, in_=tp)
                        for i in range(4):
                            nc.tensor.matmul(
                                out=gps,
                                lhsT=tsb[:, i, :],
                                rhs=tsb[:, i, :],
                                start=(kk == 0),
                                stop=(kk == 127),
                                skip_group_check=True,
                            )
                            kk += 1
                    col0 += ncols

                # x2 fp8 (needed from first C band): behind x1 on sync queue
                nc.sync.dma_start(out=p2xf8[:, 0:65, :], in_=x2f8[:, : 65 * PW])
                nc.sync.dma_start(out=p2xf8[:, 65:130, :], in_=x2f8[:, 65 * PW:])

                # diag -> norms -> rn
                gi = attn.tile([C, C], f32)
                nc.vector.tensor_mul(out=gi, in0=gps, in1=idfs)
                diag = attn.tile([C, 1], f32)
                nc.vector.reduce_sum(out=diag, in_=gi, axis=AX.X)
                norm = attn.tile([C, 1], f32)
                nc.scalar.activation(out=norm, in_=diag, func=AF.Sqrt)
                rn = attn.tile([C, 1], f32)
                nc.vector.reciprocal(out=rn, in_=norm)

                pe_warmup(42)

                # S = diag(rn) G diag(rn) via row-scale, transpose, row-scale
                s1 = attn.tile([C, C], f32)
                nc.vector.tensor_scalar_mul(out=s1, in0=gps, scalar1=rn)
                with tc.tile_pool(name="psS", bufs=1, space="PSUM") as psS:
                    s1t = psS.tile([C, C], f32)
                    nc.tensor.transpose(out=s1t, in_=s1, identity=idfs)
                    s2 = attn.tile([C, C], f32)
                    nc.vector.tensor_scalar_mul(out=s2, in0=s1t, scalar1=rn)

                pe_warmup(60)

                # E = exp(S * temp) * blockmask ; rinv = 1/rowsum(E)
                e0 = attn.tile([C, C], f32)
                nc.scalar.activation(out=e0, in_=s2, func=AF.Exp, scale=temp_ap)
                em = attn.tile([C, C], f32r)
                nc.vector.tensor_mul(out=em, in0=e0, in1=bmasks)
                rs = attn.tile([C, 1], f32)
                nc.vector.reduce_sum(out=rs, in_=em, axis=AX.X)
                rinv = attn.tile([C, 1], f32)
                nc.vector.reciprocal(out=rinv, in_=rs)

                # fused attention+conv weights, quantized to fp8*SL:
                # L_t = E diag(rinv) w1_t ; Lsc = E diag(rinv) wsc (bf16)
                ltp = []
                lsc = attn.tile([C, C], bf16, name="lsc")
                with tc.tile_pool(name="psW", bufs=2, space="PSUM") as psW:
                    for p in range(5):
                        rt = attn.tile([C, 2, C], f32r, name=f"rt{p}", tag="rt")
                        nc.vector.tensor_scalar_mul(
                            out=rt, in0=w1s[:, 2 * p : 2 * p + 2, :], scalar1=rinv
                        )
                        lps = psW.tile([C, 2, C], f32, name=f"lps{p}", tag="lps")
                        nc.tensor.matmul(
                            out=lps, lhsT=em, rhs=rt, start=True, stop=True
                        )
                        lt = attn.tile([C, 2, C], fp8, name=f"lt{p}")
                        nc.scalar.activation(
                            out=lt, in_=lps, func=AF.Copy, scale=c16_ap
                        )
                        ltp.append(lt)
                    rts = attn.tile([C, C], f32r, name="rts")
                    nc.vector.tensor_scalar_mul(out=rts, in0=wscs, scalar1=rinv)
                    lps5 = psW.tile([C, C], f32, name="lps5", tag="lps")
                    nc.tensor.matmul(out=lps5, lhsT=em, rhs=rts, start=True, stop=True)
                    nc.scalar.activation(out=lsc, in_=lps5, func=AF.Copy)
                    pe_warmup(14)
                if debug:
                    nc.gpsimd.dma_start(out=dbg_em, in_=em)
                    nc.sync.dma_start(out=dbg_rinv, in_=rinv)

            # ============ phases C/D interleaved per 4-row band ============
            # C band b writes phf8 rows 1+4b..4+4b (+pads)
            # D band b reads phf8 rows 4b..4b+5  -> needs C bands <= b+1
            with (
                tc.tile_pool(name="psC", bufs=3, space="PSUM") as psC,
                tc.tile_pool(name="psD", bufs=3, space="PSUM") as psD,
                tc.tile_pool(name="psS2", bufs=2, space="PSUM") as psS2,
                tc.tile_pool(name="otiles", bufs=2) as otiles,
            ):
                state = {}

                def emit_C(b):
                    y0 = 4 * b
                    cps = psC.tile([C, 4, C], f32)
                    for p, (dy, dx, kts) in enumerate(PAIR_GEO):
                        nc.tensor.matmul(
                            out=cps,
                            lhsT=ltp[p],
                            rhs=dr_rhs(p2xf8, y0 + dy, dx, kts),
                            start=(p == 0),
                            stop=(p == 4),
                            perf_mode=PM.DoubleRow,
                        )
                    nc.scalar.activation(
                        out=phf8[:, 1 + y0 : 5 + y0, 1 : 1 + C],
                        in_=cps,
                        func=AF.Prelu,
                        bias=b1_ap,
                        scale=cinvSL_ap,
                        alpha=SLOPE,
                    )
                    # incremental reflect pad of the left/right columns
                    nc.gpsimd.tensor_copy(
                        out=phf8[:, 1 + y0 : 5 + y0, 0:1],
                        in_=phf8[:, 1 + y0 : 5 + y0, 2:3],
                    )
                    nc.gpsimd.tensor_copy(
                        out=phf8[:, 1 + y0 : 5 + y0, PW - 1 : PW],
                        in_=phf8[:, 1 + y0 : 5 + y0, PW - 3 : PW - 2],
                    )
                    if b == 0:
                        nc.gpsimd.tensor_copy(out=phf8[:, 0:1, :], in_=phf8[:, 2:3, :])
                    if b == NB - 1:
                        nc.gpsimd.tensor_copy(
                            out=phf8[:, H + 1 : H + 2, :], in_=phf8[:, H - 1 : H, :]
                        )

                def emit_D(b):
                    y0 = 4 * b
                    if b % 2 == 0:
                        state["otile"] = otiles.tile(
                            [C, 1024], fp16, tag="otile", name="otile"
                        )
                    otile = state["otile"]
                    x1b = x1s[:, y0 * W : (y0 + 4) * W].rearrange(
                        "p (a b) -> p a b", a=4
                    )
                    dps = psD.tile([C, 4, C], f32)
                    for p, (dy, dx, kts) in enumerate(PAIR_GEO):
                        nc.tensor.matmul(
                            out=dps,
                            lhsT=w2s[:, 2 * p : 2 * p + 2, :],
                            rhs=dr_rhs(phf8, y0 + dy, dx, kts),
                            start=(p == 0),
                            stop=(p == 4),
                            perf_mode=PM.DoubleRow,
                        )
                    sps = psS2.tile([C, 4, C], f32)
                    nc.tensor.matmul(
                        out=sps,
                        lhsT=lsc,
                        rhs=x2s[:, y0 * W : (y0 + 4) * W].rearrange(
                            "p (a b) -> p a b", a=4
                        ),
                        start=True,
                        stop=True,
                    )
                    h3 = bands.tile([C, 4, C], f32)
                    nc.scalar.activation(
                        out=h3, in_=dps, func=AF.Prelu, bias=b2_ap,
                        scale=cinvW2_ap, alpha=SLOPE,
                    )
                    ob = otile[:, (b % 2) * 512 : (b % 2 + 1) * 512].rearrange(
                        "p (a b) -> p a b", a=4
                    )
                    # (sc + bsc) + x1 runs on DVE in parallel with the
                    # Prelu on ACT; h3 joins last.
                    nc.vector.scalar_tensor_tensor(
                        out=ob,
                        in0=sps,
                        scalar=bsc_ap,
                        in1=x1b,
                        op0=ALU.add,
                        op1=ALU.add,
                    )
                    nc.vector.tensor_add(out=ob, in0=ob, in1=h3)
                    if b == NB - 2:
                        # flush first half of the last pair early
                        nc.sync.dma_start(
                            out=out[:, y0 * W : (y0 + 4) * W], in_=otile[:, 0:512]
                        )
                    elif b == NB - 1:
                        nc.sync.dma_start(
                            out=out[:, y0 * W : (y0 + 4) * W], in_=otile[:, 512:1024]
                        )
                    elif b % 2 == 1:
                        nc.sync.dma_start(
                            out=out[:, (y0 - 4) * W : (y0 + 4) * W], in_=otile
                        )

                # x2 bf16 (shortcut rhs) streamed during C/D on the scalar
                # queue: chunk j needed by D(8j) at the earliest
                X2CH = [(0, 2048, 0), (2048, 3072, 1), (5120, 3072, 3),
                        (8192, 4096, 6), (12288, 4096, 10)]
                for k in range(NB + 2):
                    for c0, cn, kat in X2CH:
                        if kat == k:
                            nc.sync.dma_start(
                                out=x2s[:, c0 : c0 + cn], in_=x2h[:, c0 : c0 + cn]
                            )
                    if k < NB:
                        emit_C(k)
                    if k >= 2:
                        emit_D(k - 2)
                        if debug and k == NB + 1:
                            dphf = bands.tile([C, H + 2, PW], f32, bufs=1)
                            nc.vector.tensor_copy(out=dphf, in_=phf8)
                            nc.gpsimd.dma_start(out=dbg_ph, in_=dphf)

    nc.compile()
    return nc


def _prep_consts(temperature, conv1_w, conv2_w, dw_w, conv1_b, conv2_b, dw_b, sc_b, sc_w):
    f32 = np.float32
    f8 = ml_dtypes.float8_e4m3
    conv1_w = np.asarray(conv1_w, f32)
    conv2_w = np.asarray(conv2_w, f32)
    dw_w = np.asarray(dw_w, f32)
    sc_w = np.asarray(sc_w, f32)
    # conv1 taps as lhsT in DoubleRow pair order (col j = TAP_ORDER[j])
    w1 = conv1_w.transpose(1, 2, 3, 0).reshape(C, 9, C)  # [ci, t, co]
    wc1 = np.zeros((C, 10, C), f32)
    A2 = conv2_w[:, :, 0, 0]                      # [co, ci]
    Dw = dw_w[:, 0, :, :].reshape(C, 9)           # [co, t]
    w2 = np.einsum("oc,ot->cto", A2, Dw)          # [ci, t, co]
    wc2 = np.zeros((C, 10, C), f32)
    for j, t in enumerate(TAP_ORDER):
        if t is not None:
            wc1[:, j, :] = w1[:, t, :]
            wc2[:, j, :] = w2[:, t, :] * SW2
    wsc = np.ascontiguousarray(sc_w[:, :, 0, 0].T.astype(f32))
    b2p = np.asarray(dw_b, f32) + np.asarray(conv2_b, f32) * Dw.sum(axis=1)
    temp_b = np.repeat(np.asarray(temperature, f32).reshape(HEADS), HEAD_C)
    scl = np.ascontiguousarray(
        np.stack(
            [np.asarray(conv1_b, f32), b2p, np.asarray(sc_b, f32), temp_b,
             np.full(C, SL, f32), np.full(C, 1.0 / SL, f32),
             np.full(C, 1.0 / SW2, f32)], axis=1
        )
    )  # [128, 7]
    bmask = np.kron(np.eye(HEADS, dtype=f32), np.ones((HEAD_C, HEAD_C), f32))
    idf = np.eye(C, dtype=f32)
    idb = np.eye(C, dtype=ml_dtypes.bfloat16)
    return dict(
        wc1=np.ascontiguousarray(wc1.astype(ml_dtypes.bfloat16)),
        wc2=np.ascontiguousarray(wc2.astype(f8)),
        wsc=wsc, scl=scl,
        bmask=np.ascontiguousarray(bmask),
        idf=np.ascontiguousarray(idf),
        idb=np.ascontiguousarray(idb),
    )


def kernel(
    x1, x2, temperature, conv1_w, conv1_b, conv2_w, conv2_b, dw_w, dw_b, sc_w, sc_b
):
    from concourse.bass_utils import run_bass_kernel_spmd

    if "nc" not in _cache:
        _cache["nc"] = _build_program()
    nc = _cache["nc"]

    f8 = ml_dtypes.float8_e4m3
    bf = ml_dtypes.bfloat16
    x1 = np.asarray(x1, np.float32)
    x2 = np.asarray(x2, np.float32)
    consts = _prep_consts(
        temperature, conv1_w, conv2_w, dw_w, conv1_b, conv2_b, dw_b, sc_b, sc_w
    )
    # host-prepadded fp8 x2 (zero borders)
    x2p = np.zeros((B, H + 2, PW, C), np.float32)
    x2p[:, 1 : H + 1, 1 : W + 1, :] = x2.transpose(0, 2, 3, 1)
    x2p = np.ascontiguousarray(x2p.transpose(0, 3, 1, 2)).astype(f8)

    in_maps = []
    for b in range(B):
        m = dict(consts)
        m["x1h"] = np.ascontiguousarray(x1[b].reshape(C, HW).astype(bf))
        m["x2h"] = np.ascontiguousarray(x2[b].reshape(C, HW).astype(bf))
        m["x2f8"] = np.ascontiguousarray(x2p[b].reshape(C, (H + 2) * PW))
        in_maps.append(m)

    res = run_bass_kernel_spmd(nc, in_maps, core_ids=list(range(B)))
    outs = [res.results[b]["out"].astype(np.float32).reshape(C, H, W)
            for b in range(B)]
    return np.stack(outs, axis=0)
